# revision 15
# baseline (speedup 1.0000x reference)
"""Trainium2 Bass kernel for nn_DecoderRNN (show-attend-tell decoder).

Strategy (8 NeuronCores):
  Kernel 1 — batch-parallel recurrence (4 samples/core, no collectives):
    full Bahdanau attention + LSTM cell, 20 steps. All matmuls arranged so
    the batch (4) is the matmul N-dim; weights are host-pre-transposed and
    cast to bf16; sigmoid is computed via tanh (single ACT table set);
    biases enter as K=1 ones-row matmul chunks; the attention-weighted
    encoding (awe) is one accumulated matmul with a block-diagonal alpha
    lhsT against the SBUF-resident padded encoder.
  Host — gathers per-step hidden states from all cores, reshards by vocab.
  Kernel 2 — vocab-parallel fc: [640,513] @ [513,3750] per core.
"""
import os

os.environ.setdefault("JAX_PLATFORMS", "cpu")

from contextlib import ExitStack

import numpy as np
import ml_dtypes

import concourse.bass as bass
import concourse.tile as tile
from concourse import bacc
from concourse import mybir, masks
from concourse.bass_utils import run_bass_kernel_spmd

F32 = mybir.dt.float32
BF16 = mybir.dt.bfloat16
AF = mybir.ActivationFunctionType
ALU = mybir.AluOpType
BF = ml_dtypes.bfloat16

# Problem dims (hardcoded per spec)
B, P, ENC, DEC, ATT, E, V, L = 32, 196, 2048, 512, 512, 256, 30000, 21
T = L - 1              # 20 decode steps
NCORES = 8
BL = B // NCORES       # 4 samples per core
PPAD = 256             # p padded to 2 chunks of 128
VS = V // NCORES       # 3750 vocab shard

_CACHE = {}


# ----------------------------------------------------------------------------
# Kernel 1: recurrence
# ----------------------------------------------------------------------------
def build_kernel1(debug=False):
    nc = bacc.Bacc("TRN2", target_bir_lowering=False, debug=False)
    dt = nc.dram_tensor
    # inputs (per-core shards / replicated weights), all host-prepped
    encP = dt("encP", [BL, 2, 128, ENC], BF16, kind="ExternalInput")
    encT = dt("encT", [16, 128, BL * P], BF16, kind="ExternalInput")
    WeT = dt("WeT", [16, 128, ATT], BF16, kind="ExternalInput")
    WdT05 = dt("WdT05", [4, 128, ATT], BF16, kind="ExternalInput")
    beT = dt("beT", [1, ATT], BF16, kind="ExternalInput")
    blkWf = dt("blkWf", [128, 16, BL], BF16, kind="ExternalInput")
    WT = dt("WT", [23, 128, 4 * DEC], BF16, kind="ExternalInput")
    Wh0T = dt("Wh0T", [17, 128, DEC], BF16, kind="ExternalInput")
    Wc0T = dt("Wc0T", [17, 128, DEC], BF16, kind="ExternalInput")
    embT = dt("embT", [2, 128, T, BL], BF16, kind="ExternalInput")
    # outputs
    alphas_o = dt("alphas", [BL, T, P], F32, kind="ExternalOutput")
    hT2_o = dt("hT2", [T, 128, 16], BF16, kind="ExternalOutput")
    if debug:
        d_mean = dt("d_mean", [32, 4 * DEC], BF16, kind="ExternalOutput")
        d_h0T = dt("d_h0T", [128, 16], BF16, kind="ExternalOutput")
        d_c0 = dt("d_c0", [BL, DEC], F32, kind="ExternalOutput")
        d_att1T = dt("d_att1T", [128, 4 * BL * P], BF16, kind="ExternalOutput")
        d_att2T = dt("d_att2T", [128, 16], F32, kind="ExternalOutput")
        d_e = dt("d_e", [BL, P], F32, kind="ExternalOutput")
        d_awe = dt("d_awe", [32, 4 * DEC], BF16, kind="ExternalOutput")
        d_gates = dt("d_gates", [BL, 4 * DEC], F32, kind="ExternalOutput")

    with tile.TileContext(nc) as tc, ExitStack() as ctx:
        G = 4 * DEC  # 2048

        const = ctx.enter_context(tc.tile_pool(name="const", bufs=1))
        state = ctx.enter_context(tc.tile_pool(name="state", bufs=1))
        psum_p = ctx.enter_context(tc.tile_pool(name="psum_p", bufs=1, space="PSUM"))
        psum_t = ctx.enter_context(tc.tile_pool(name="psum_t", bufs=2, space="PSUM"))
        work = ctx.enter_context(tc.tile_pool(name="work", bufs=2))
        pw = ctx.enter_context(tc.tile_pool(name="pw", bufs=2))

        ident = const.tile([128, 128], F32)
        masks.make_identity(nc, ident[:])
        ones_bf = const.tile([1, BL], BF16)
        nc.gpsimd.memset(ones_bf[:], 1.0)

        # big persistent psum: awe + gates (+ mean/att1 in precompute)
        ps_big = psum_p.tile([128, G], F32)
        nc.vector.memset(ps_big[:], 0.0)

        # --- resident tensors ---
        encP_sb = const.tile([128, BL * 2, ENC], BF16)
        nc.sync.dma_start(encP_sb[:], encP.ap().rearrange("b c p e -> p (b c) e"))
        WdT05_sb = const.tile([128, 4, ATT], BF16)
        nc.sync.dma_start(WdT05_sb[:], WdT05.ap().rearrange("c p n -> p c n"))
        beT_sb = const.tile([1, ATT], BF16)
        nc.sync.dma_start(beT_sb[:], beT.ap())
        blkWf_sb = const.tile([128, 16, BL], BF16)
        nc.sync.dma_start(blkWf_sb[:], blkWf.ap())
        embT_sb = const.tile([128, 2, T, BL], BF16)
        nc.sync.dma_start(embT_sb[:], embT.ap().rearrange("c p t b -> p c t b"))

        att1T_sb = const.tile([128, 4, BL * P], BF16)
        blkdiag = state.tile([128, 2 * BL, BL], BF16)  # (b,pc) chunks x cols b
        alphaT_pad = state.tile([128, 2, BL], BF16)
        nc.vector.memset(alphaT_pad[:], 0.0)
        awe_sb = state.tile([32, G], BF16)      # rows 0:BL valid
        nc.vector.memset(awe_sb[:], 0.0)
        aweT = state.tile([128, 16, 32], BF16)  # cols 0:BL valid
        hT2_sb = state.tile([128, 4, BL], BF16)
        t2c_sb = state.tile([BL, DEC], F32)
        att2T_sb = state.tile([128, 16], F32)

        # --- precompute: blkdiag <- 1/196 diagonal (for mean pooling) ---
        nc.vector.memset(blkdiag[:], 0.0)
        for b in range(BL):
            nc.vector.memset(blkdiag[:, 2 * b, b:b + 1], 1.0 / P)
            nc.vector.memset(blkdiag[0:P - 128, 2 * b + 1, b:b + 1], 1.0 / P)

        # mean_enc = blkdiag_mean.T @ encP  -> ps_big[0:BL, :]
        for n in range(4):
            for k in range(2 * BL):
                nc.tensor.matmul(
                    ps_big[0:BL, n * 512:(n + 1) * 512],
                    blkdiag[:, k, :], encP_sb[:, k, n * 512:(n + 1) * 512],
                    start=(k == 0), stop=(k == 2 * BL - 1))
        # -> awe_sb (bf16) -> XBAR transpose -> aweT (= mencT)
        for n in range(4):
            nsl = slice(n * 512, (n + 1) * 512)
            if n % 2 == 0:
                nc.vector.tensor_copy(awe_sb[0:BL, nsl], ps_big[0:BL, nsl])
            else:
                nc.scalar.copy(awe_sb[0:BL, nsl], ps_big[0:BL, nsl])
        nc.sync.dma_start(aweT[:], awe_sb[:], transpose=True)

        with tc.tile_pool(name="pre_h0", bufs=1) as pre_h0, \
                tc.tile_pool(name="pre_h0p", bufs=1, space="PSUM") as pre_h0p:
            Wh0T_sb = pre_h0.tile([128, 17, DEC], BF16)
            nc.sync.dma_start(Wh0T_sb[:], Wh0T.ap().rearrange("c p n -> p c n"))
            Wc0T_sb = pre_h0.tile([128, 17, DEC], BF16)
            nc.sync.dma_start(Wc0T_sb[:], Wc0T.ap().rearrange("c p n -> p c n"))
            h0ps = pre_h0p.tile([BL, DEC], F32)
            c0ps = pre_h0p.tile([BL, DEC], F32)
            for k in range(16):
                nc.tensor.matmul(h0ps[:], aweT[:, k, 0:BL], Wh0T_sb[:, k, :],
                                 start=(k == 0), stop=False)
                nc.tensor.matmul(c0ps[:], aweT[:, k, 0:BL], Wc0T_sb[:, k, :],
                                 start=(k == 0), stop=False)
            nc.tensor.matmul(h0ps[:], ones_bf[:], Wh0T_sb[0:1, 16, :],
                             start=False, stop=True)
            nc.tensor.matmul(c0ps[:], ones_bf[:], Wc0T_sb[0:1, 16, :],
                             start=False, stop=True)
            # c state (= 2c) stays batch-major
            nc.vector.tensor_copy(t2c_sb[:], c0ps[:])
            # h state -> transposed via PE (in must be SBUF)
            h0sb = pre_h0.tile([BL, DEC], F32)
            nc.vector.tensor_copy(h0sb[:], h0ps[:])
            hTps = psum_t.tile([128, 16], F32, tag="pt")
            for hc in range(4):
                nc.tensor.transpose(hTps[:, hc * BL:(hc + 1) * BL],
                                    h0sb[:, hc * 128:(hc + 1) * 128],
                                    ident[0:BL, 0:BL])
            nc.vector.tensor_copy(hT2_sb[:].rearrange("p a b -> p (a b)"), hTps[:])

        if debug:
            nc.sync.dma_start(d_mean.ap(), awe_sb[:])
            nc.sync.dma_start(d_h0T.ap(), hT2_sb[:].rearrange("p a b -> p (a b)"))
            nc.sync.dma_start(d_c0.ap(), t2c_sb[:])

        # --- att1T = We @ enc.T  (a on partitions, (b,p) free) ---
        with tc.tile_pool(name="pre_att1", bufs=1) as pre_a1, \
                tc.tile_pool(name="pre_a1p", bufs=1, space="PSUM") as pre_a1p:
            encT_sb = pre_a1.tile([128, 16, BL * P], BF16)
            nc.sync.dma_start(encT_sb[:], encT.ap().rearrange("c p n -> p c n"))
            WeT_sb = pre_a1.tile([128, 16, ATT], BF16)
            nc.sync.dma_start(WeT_sb[:], WeT.ap().rearrange("c p n -> p c n"))
            BP = BL * P  # 784
            for at in range(4):
                a1ps = pre_a1p.tile([128, BP], F32, tag="a1ps")
                for n in range(2):
                    nsl = slice(n * 512, min((n + 1) * 512, BP))
                    for k in range(16):
                        nc.tensor.matmul(
                            a1ps[:, nsl],
                            WeT_sb[:, k, at * 128:(at + 1) * 128],
                            encT_sb[:, k, nsl],
                            start=(k == 0), stop=(k == 15))
                if at % 2 == 0:
                    nc.vector.tensor_copy(att1T_sb[:, at, :], a1ps[:])
                else:
                    nc.scalar.copy(att1T_sb[:, at, :], a1ps[:])

        if debug:
            nc.sync.dma_start(d_att1T.ap(),
                              att1T_sb[:].rearrange("p a n -> p (a n)"))

        wt_pool = ctx.enter_context(tc.tile_pool(name="wt", bufs=1))
        WT_sb = wt_pool.tile([128, 23, G], BF16)
        for k in range(23):
            nc.sync.dma_start(WT_sb[:, k, :], WT.ap()[k])

        # ------------------------- decode steps -------------------------
        for t in range(T):
            # att2T[a, b] = 0.5*Wd @ (2h) + be   [128, 4at*4b] psum
            a2ps = psum_t.tile([128, 16], F32, tag="pt")
            for at in range(4):
                osl = a2ps[:, at * BL:(at + 1) * BL]
                for hc in range(4):
                    nc.tensor.matmul(osl, WdT05_sb[:, hc, at * 128:(at + 1) * 128],
                                     hT2_sb[:, hc, :], start=(hc == 0), stop=False)
                nc.tensor.matmul(osl, beT_sb[0:1, at * 128:(at + 1) * 128],
                                 ones_bf[:], start=False, stop=True)
            nc.vector.tensor_copy(att2T_sb[:], a2ps[:])
            if debug and t == 0:
                nc.sync.dma_start(d_att2T.ap(), att2T_sb[:])

            # combined = relu(att1T + att2T)  -> e = blkWf.T @ comb (accum 16)
            eps = psum_t.tile([BL, P], F32, tag="pt")
            for b in range(BL):
                comb = work.tile([128, 4, P], BF16, tag="comb")
                for ac in range(4):
                    idx = ac * BL + b
                    nc.vector.tensor_scalar(
                        comb[:, ac, :], att1T_sb[:, ac, b * P:(b + 1) * P],
                        att2T_sb[:, idx:idx + 1], 0.0, op0=ALU.add, op1=ALU.max)
                for ac in range(4):
                    nc.tensor.matmul(eps[:], blkWf_sb[:, b * 4 + ac, :],
                                     comb[:, ac, :],
                                     start=(b == 0 and ac == 0),
                                     stop=(b == BL - 1 and ac == 3))

            if debug and t == 0:
                dbg_e = pw.tile([BL, P], F32, tag="dbg_e")
                nc.vector.tensor_copy(dbg_e[:], eps[:])
                nc.sync.dma_start(d_e.ap(), dbg_e[:])
            # softmax over p (free dim)
            nmx = pw.tile([BL, 1], F32, tag="nmx")
            nc.vector.tensor_reduce(nmx[:], eps[:], axis=mybir.AxisListType.X,
                                    op=ALU.max, negate=True)
            exps = pw.tile([BL, P], F32, tag="exps")
            nc.scalar.activation(exps[:], eps[:], AF.Exp, bias=nmx[:], scale=1.0)
            ssum = pw.tile([BL, 1], F32, tag="ssum")
            nc.vector.tensor_reduce(ssum[:], exps[:], axis=mybir.AxisListType.X,
                                    op=ALU.add)
            rec = pw.tile([BL, 1], F32, tag="rec")
            nc.vector.reciprocal(rec[:], ssum[:])
            alpha_t = pw.tile([BL, P], F32, tag="alpha_t")
            alpha = alpha_t[:]
            nc.vector.tensor_scalar(alpha, exps[:], rec[:], None, op0=ALU.mult)
            nc.sync.dma_start(alphas_o.ap()[:, t, :], alpha)

            # alphaT (padded to 256) -> blkdiag scatter
            aTx = psum_t.tile([128, 2 * BL], F32, tag="pt")
            nc.tensor.transpose(aTx[:, 0:BL], alpha[:, 0:128], ident[0:BL, 0:BL])
            nc.tensor.transpose(aTx[0:P - 128, BL:2 * BL], alpha[:, 128:P],
                                ident[0:BL, 0:BL])
            nc.vector.tensor_copy(alphaT_pad[:, 0, :], aTx[:, 0:BL])
            nc.vector.tensor_copy(alphaT_pad[0:P - 128, 1, :],
                                  aTx[0:P - 128, BL:2 * BL])
            for b in range(BL):
                nc.vector.tensor_copy(blkdiag[:, 2 * b:2 * b + 2, b:b + 1],
                                      alphaT_pad[:, :, b:b + 1])

            # awe = blkdiag.T @ encP -> ps_big[0:BL, :]
            for n in range(4):
                for k in range(2 * BL):
                    nc.tensor.matmul(
                        ps_big[0:BL, n * 512:(n + 1) * 512],
                        blkdiag[:, k, :], encP_sb[:, k, n * 512:(n + 1) * 512],
                        start=(k == 0), stop=(k == 2 * BL - 1))
            for n in range(4):
                nsl = slice(n * 512, (n + 1) * 512)
                if n % 2 == 0:
                    nc.vector.tensor_copy(awe_sb[0:BL, nsl], ps_big[0:BL, nsl])
                else:
                    nc.scalar.copy(awe_sb[0:BL, nsl], ps_big[0:BL, nsl])
            nc.sync.dma_start(aweT[:], awe_sb[:], transpose=True)
            if debug and t == 0:
                nc.sync.dma_start(d_awe.ap(), awe_sb[:])

            # gates = [embT | aweT | hT2 | ones] @ WT -> ps_big[0:BL, :]
            def lhsT_chunk(k):
                if k < 2:
                    return embT_sb[:, k, t, :]
                if k < 18:
                    return aweT[:, k - 2, 0:BL]
                if k < 22:
                    return hT2_sb[:, k - 18, :]
                return ones_bf[:]

            for n in range(4):
                nsl = slice(n * 512, (n + 1) * 512)
                for k in range(23):
                    rhs = WT_sb[:, k, nsl] if k < 22 else WT_sb[0:1, 22, nsl]
                    nc.tensor.matmul(ps_big[0:BL, nsl], lhsT_chunk(k), rhs,
                                     start=(k == 0), stop=(k == 22))

            if debug and t == 0:
                dbg_g = pw.tile([BL, 4 * DEC], F32, tag="dbg_g")
                for _n in range(4):
                    _sl = slice(_n * 512, (_n + 1) * 512)
                    nc.vector.tensor_copy(dbg_g[:, _sl], ps_big[0:BL, _sl])
                nc.sync.dma_start(d_gates.ap(), dbg_g[:])
            # LSTM pointwise: sigma(x) = 0.5*tanh(x/2)+0.5 ; states are 2h, 2c
            ti = pw.tile([BL, DEC], F32, tag="ti")
            tf = pw.tile([BL, DEC], F32, tag="tf")
            tg = pw.tile([BL, DEC], F32, tag="tg")
            to = pw.tile([BL, DEC], F32, tag="to")
            nc.scalar.activation(ti[:], ps_big[0:BL, 0:512], AF.Tanh, scale=0.5)
            nc.scalar.activation(tf[:], ps_big[0:BL, 512:1024], AF.Tanh, scale=0.5)
            nc.scalar.activation(tg[:], ps_big[0:BL, 1024:1536], AF.Tanh, scale=1.0)
            nc.scalar.activation(to[:], ps_big[0:BL, 1536:2048], AF.Tanh, scale=0.5)
            s1 = pw.tile([BL, DEC], F32, tag="s1")
            nc.vector.scalar_tensor_tensor(s1[:], tf[:], 1.0, t2c_sb[:],
                                           op0=ALU.add, op1=ALU.mult)
            s2 = pw.tile([BL, DEC], F32, tag="s2")
            nc.vector.scalar_tensor_tensor(s2[:], ti[:], 1.0, tg[:],
                                           op0=ALU.add, op1=ALU.mult)
            # 2*c_new = 0.5*(tf+1)*(2c) + (ti+1)*tg
            nc.vector.scalar_tensor_tensor(t2c_sb[:], s1[:], 0.5, s2[:],
                                           op0=ALU.mult, op1=ALU.add)
            tc_ = pw.tile([BL, DEC], F32, tag="tc_")
            nc.scalar.activation(tc_[:], t2c_sb[:], AF.Tanh, scale=0.5)
            h2 = pw.tile([BL, DEC], F32, tag="h2")
            nc.vector.scalar_tensor_tensor(h2[:], to[:], 1.0, tc_[:],
                                           op0=ALU.add, op1=ALU.mult)

            # hT2 <- transpose(h2)
            hTps = psum_t.tile([128, 16], F32, tag="pt")
            for hc in range(4):
                nc.tensor.transpose(hTps[:, hc * BL:(hc + 1) * BL],
                                    h2[:, hc * 128:(hc + 1) * 128],
                                    ident[0:BL, 0:BL])
            nc.vector.tensor_copy(hT2_sb[:].rearrange("p a b -> p (a b)"), hTps[:])
            nc.sync.dma_start(hT2_o.ap()[t], hT2_sb[:].rearrange("p a b -> p (a b)"))

    nc.compile()
    return nc


# ----------------------------------------------------------------------------
# Kernel 2: fc over vocab shard
# ----------------------------------------------------------------------------
def build_kernel2():
    nc = bacc.Bacc("TRN2", target_bir_lowering=False, debug=False)
    dt = nc.dram_tensor
    M = T * B  # 640
    hT = dt("hT", [4, 128, M], BF16, kind="ExternalInput")
    Wfc = dt("Wfc", [4, 128, VS], BF16, kind="ExternalInput")
    bfc = dt("bfc", [1, VS], BF16, kind="ExternalInput")
    preds = dt("preds", [M, VS], F32, kind="ExternalOutput")

    with tile.TileContext(nc) as tc, ExitStack() as ctx:
        const = ctx.enter_context(tc.tile_pool(name="const", bufs=1))
        work = ctx.enter_context(tc.tile_pool(name="work", bufs=3))
        psum = ctx.enter_context(tc.tile_pool(name="psum", bufs=4, space="PSUM"))

        hT_sb = const.tile([128, 4, M], BF16)
        nc.sync.dma_start(hT_sb[:], hT.ap().rearrange("c p n -> p c n"))
        bfc_sb = const.tile([1, VS], BF16)
        nc.sync.dma_start(bfc_sb[:], bfc.ap())
        ones_bf = const.tile([1, 128], BF16)
        nc.gpsimd.memset(ones_bf[:], 1.0)
        Wfc_sb = const.tile([128, 4, VS], BF16)
        for k in range(4):
            nc.sync.dma_start(Wfc_sb[:, k, :], Wfc.ap()[k])

        NCH = (VS + 511) // 512  # 8
        for mt in range(5):
            msl = slice(mt * 128, (mt + 1) * 128)
            for n in range(NCH):
                nsl = slice(n * 512, min((n + 1) * 512, VS))
                w = nsl.stop - nsl.start
                ps = psum.tile([128, 512], F32, tag="ps")
                for k in range(4):
                    nc.tensor.matmul(ps[:, 0:w], hT_sb[:, k, msl],
                                     Wfc_sb[:, k, nsl], start=(k == 0), stop=False)
                nc.tensor.matmul(ps[:, 0:w], ones_bf[:], bfc_sb[:, nsl],
                                 start=False, stop=True)
                ob = work.tile([128, 512], F32, tag="ob")
                if n % 2 == 0:
                    nc.vector.tensor_copy(ob[:, 0:w], ps[:, 0:w])
                else:
                    nc.scalar.copy(ob[:, 0:w], ps[:, 0:w])
                nc.sync.dma_start(preds.ap()[msl, nsl], ob[:, 0:w])
    nc.compile()
    return nc


# ----------------------------------------------------------------------------
# Host-side prep / glue
# ----------------------------------------------------------------------------
def _prep_shared(inp):
    f32 = lambda x: np.asarray(x, np.float32)
    W_ih, W_hh = f32(inp["W_ih"]), f32(inp["W_hh"])
    b_ih, b_hh = f32(inp["b_ih"]), f32(inp["b_hh"])
    We, be = f32(inp["We_att"]), f32(inp["be_att"])
    Wd, Wf = f32(inp["Wd_att"]), f32(inp["Wf_att"])
    Wh0, bh0 = f32(inp["W_h0"]), f32(inp["b_h0"])
    Wc0, bc0 = f32(inp["W_c0"]), f32(inp["b_c0"])

    d = {}
    d["WeT"] = We.T.reshape(16, 128, ATT).astype(BF)
    d["WdT05"] = (0.5 * Wd.T).reshape(4, 128, ATT).astype(BF)
    d["beT"] = be.reshape(1, ATT).astype(BF)
    blkWf = np.zeros((128, 16, BL), np.float32)
    for b in range(BL):
        for ac in range(4):
            blkWf[:, b * 4 + ac, b] = Wf[0, ac * 128:(ac + 1) * 128]
    d["blkWf"] = blkWf.astype(BF)
    WTr = np.concatenate(
        [W_ih.T, 0.5 * W_hh.T, (b_ih + b_hh)[None, :]], axis=0)  # [2817, 2048]
    WT = np.zeros((23 * 128, 4 * DEC), np.float32)
    WT[:2817] = WTr
    d["WT"] = WT.reshape(23, 128, 4 * DEC).astype(BF)
    for nm, W0, b0 in (("Wh0T", Wh0, bh0), ("Wc0T", Wc0, bc0)):
        Wx = np.zeros((17 * 128, DEC), np.float32)
        Wx[:2048] = 2.0 * W0.T
        Wx[2048] = 2.0 * b0
        d[nm] = Wx.reshape(17, 128, DEC).astype(BF)
    return d


def _prep_core(inp, i, emb_gathered):
    enc = np.asarray(inp["encoder_out"], np.float32)[i * BL:(i + 1) * BL]
    d = {}
    pad = np.zeros((BL, PPAD, ENC), np.float32)
    pad[:, :P] = enc
    d["encP"] = pad.reshape(BL, 2, 128, ENC).astype(BF)
    d["encT"] = np.ascontiguousarray(enc.transpose(2, 0, 1)).reshape(
        16, 128, BL * P).astype(BF)
    et = emb_gathered[i * BL:(i + 1) * BL]          # [BL, T, E]
    d["embT"] = np.ascontiguousarray(et.transpose(2, 1, 0)).reshape(
        2, 128, T, BL).astype(BF)
    return d


def kernel(**inputs):
    if "k1" not in _CACHE:
        _CACHE["k1"] = build_kernel1()
        _CACHE["k2"] = build_kernel2()
    nc1, nc2 = _CACHE["k1"], _CACHE["k2"]

    emb = np.asarray(inputs["emb"], np.float32)
    captions = np.asarray(inputs["captions"], np.int64)
    emb_gathered = emb[captions[:, :T]]             # [B, T, E]

    shared = _prep_shared(inputs)
    in_maps1 = []
    for i in range(NCORES):
        m = dict(shared)
        m.update(_prep_core(inputs, i, emb_gathered))
        in_maps1.append(m)

    r1 = run_bass_kernel_spmd(nc1, in_maps1, core_ids=list(range(NCORES)))
    t1 = r1.exec_time_ns

    # assemble h (state is 2h; W_fc scaled by 0.5 on host)
    H = np.zeros((4, 128, T, B), np.float32)
    alphas = np.zeros((B, T, P), np.float32)
    for i in range(NCORES):
        out = r1.results[i]
        arr = np.asarray(out["hT2"], BF).astype(np.float32)  # [T,128,16]
        H[:, :, :, i * BL:(i + 1) * BL] = arr.reshape(T, 128, 4, BL).transpose(
            2, 1, 0, 3)
        alphas[i * BL:(i + 1) * BL] = np.asarray(out["alphas"], np.float32)
    hT = H.reshape(4, 128, T * B).astype(BF)

    Wfc05 = (0.5 * np.asarray(inputs["W_fc"], np.float32).T)  # [512, V]
    bfc = np.asarray(inputs["b_fc"], np.float32)
    in_maps2 = []
    for j in range(NCORES):
        vsl = slice(j * VS, (j + 1) * VS)
        in_maps2.append({
            "hT": hT,
            "Wfc": np.ascontiguousarray(Wfc05[:, vsl]).reshape(
                4, 128, VS).astype(BF),
            "bfc": bfc[vsl].reshape(1, VS).astype(BF),
        })
    r2 = run_bass_kernel_spmd(nc2, in_maps2, core_ids=list(range(NCORES)))
    t2 = r2.exec_time_ns

    preds = np.zeros((B, T, V), np.float32)
    for j in range(NCORES):
        pj = np.asarray(r2.results[j]["preds"], np.float32)  # [640, VS]
        preds[:, :, j * VS:(j + 1) * VS] = pj.reshape(T, B, VS).transpose(1, 0, 2)

    kernel.exec_time_ns = (t1 or 0) + (t2 or 0)
    kernel.exec_times = (t1, t2)
    return preds, alphas
